# revision 25
# baseline (speedup 1.0000x reference)
"""LightGCN (3-layer propagation + BPR loss) on 8 Trainium2 NeuronCores.

Strategy (dst-sharded ELL):
  - Nodes (100k users + 50k items, padded to 150528) are permuted:
    loss-needed nodes first, then by degree descending, dealt round-robin
    into 8 cores x 147 tiles x 128 lanes. Global permuted id =
    core*18816 + lane*147 + slot.
  - The propagation table stores t = dinv * h, so per-edge weights vanish:
    h_new[d] = dinv[d] * sum_{e: dst=d} t[src[e]];  t_new = dinv[d] * h_new.
  - Each core owns its 147 dst tiles. Per tile: ELL layout, k_i slots per
    lane (k_i = max degree in tile across cores), gathered with one
    indirect DMA per slot-column (128 rows each) from the full table in
    DRAM, then one strided DVE reduce -> [128, 64].
  - Per layer the table is rebuilt from the 8 slices with TWO chunked
    AllGathers (slot ranges BNDS): chunk 0 is issued mid-layer so the
    collective (running on TOPSP/SDMA silicon) overlaps the remaining
    tiles' gather+reduce; only the small chunk-1 tail stays exposed.
    Table rows are laid out chunk-major (_trow); gather indices are
    precomputed for that layout. Layer 3 computes only the tiles that
    the BPR loss actually reads (pruned via the permutation).
  - Final: mini-AllGather of the needed acc slots, per-core gathers of
    user/pos/neg rows, dot products, softplus on ACT; host sums partials.
"""
import sys

sys.path.insert(0, "/opt/trn_rl_repo")

import numpy as np

import concourse.bass as bass
import concourse.mybir as mybir
import concourse.tile as tile
from concourse.bass_utils import run_bass_kernel_spmd

NU, NI, D = 100000, 50000, 64
N = NU + NI
NL = 3
LW = 1e-4
B = 8192
C = 8                       # cores
TPC = 147                   # tiles (slots) per core
P = 128                     # lanes
NPC = TPC * P               # nodes per core = 18816
NPAD = C * NPC              # 150528
BPC = B // C                # samples per core = 1024
SCOL = BPC // P             # sample columns = 8
BNDS = [0, 96, 116, TPC]    # AllGather chunk boundaries (slot ranges)


def _trow(pid):
    """Table row of node pid under the chunked-AllGather layout: chunk c
    covers slots [BNDS[c], BNDS[c+1]) of all cores, concatenated per core."""
    c = pid // NPC
    rem = pid % NPC
    q = rem // TPC
    i = rem % TPC
    row = np.zeros_like(pid)
    base = 0
    for lo, hi in zip(BNDS[:-1], BNDS[1:]):
        w = hi - lo
        sel = (i >= lo) & (i < hi)
        row = np.where(sel, base + c * (P * w) + q * w + (i - lo), row)
        base += C * P * w
    return row


def _split_multi_waits(nc):
    """This walrus build allows one sync-wait per instruction; move extras
    onto same-engine NoOps placed immediately before."""
    n = 0
    for func in nc.m.functions:
        for bb in func.blocks:
            out = []
            for inst in bb.instructions:
                si = inst.sync_info
                if si is not None and len(si.on_wait) > 1:
                    waits = list(si.on_wait)
                    for w in waits[:-1]:
                        nop = mybir.InstNoOp(name=f"{inst.name}-w{n}", ins=[], outs=[])
                        nop.engine = inst.engine
                        nop.sync_info = mybir.SyncInfo(on_wait=[w], on_update=[])
                        out.append(nop)
                        n += 1
                    inst.sync_info = mybir.SyncInfo(
                        on_wait=[waits[-1]], on_update=list(si.on_update)
                    )
                out.append(inst)
            if n:
                bb.instructions = out
    return n


def _prep(Gu, Gi, edge_user, edge_item, user, pos, neg):
    eu = np.asarray(edge_user).astype(np.int64).ravel()
    ei = np.asarray(edge_item).astype(np.int64).ravel()
    user = np.asarray(user).astype(np.int64).ravel()
    pos = np.asarray(pos).astype(np.int64).ravel()
    neg = np.asarray(neg).astype(np.int64).ravel()
    Gu = np.asarray(Gu, dtype=np.float32)
    Gi = np.asarray(Gi, dtype=np.float32)

    src = np.concatenate([eu, ei + NU])
    dst = np.concatenate([ei + NU, eu])
    deg = np.bincount(dst, minlength=N).astype(np.float32)
    dinv = np.zeros(N, np.float32)
    nz = deg > 0
    dinv[nz] = (1.0 / np.sqrt(deg[nz])).astype(np.float32)

    x = np.concatenate([Gu, Gi], axis=0)                      # [N, D]

    # ---- node permutation: needed-first, then degree desc ----
    needed = np.zeros(NPAD, bool)
    needed[user] = True
    needed[pos + NU] = True
    needed[neg + NU] = True
    deg_pad = np.concatenate([deg, np.zeros(NPAD - N, np.float32)])
    # layer-3 only reads t2 of sources of edges into needed nodes; cluster
    # those so layer 2 can skip tiles nobody reads (stale t1 left in the
    # flush buffer for skipped slots is never gathered).
    l3src = np.zeros(NPAD, bool)
    l3src[src[needed[dst]]] = True
    # sort key: needed desc, l3-source desc, degree desc
    order = np.lexsort((-deg_pad, ~l3src[:NPAD], ~needed[:NPAD]))
    r = np.arange(NPAD)
    t_rank = r // P
    lane = r % P
    core_of_rank = t_rank % C
    slot_of_rank = t_rank // C
    pid_of_rank = core_of_rank * NPC + lane * TPC + slot_of_rank
    pid = np.empty(NPAD, np.int64)
    pid[order] = pid_of_rank

    n_need = int(needed.sum())
    need_tiles = (n_need + P - 1) // P
    need_slots = (need_tiles + C - 1) // C                    # per-core slots for layer 3
    need_slots = max(need_slots, 1)
    n_active = int((needed | l3src).sum())
    act_tiles = (n_active + P - 1) // P
    mid_slots = min(TPC, max((act_tiles + C - 1) // C, need_slots))

    # ---- per-node info in permuted space ----
    deg_perm = np.zeros(NPAD, np.float32)
    deg_perm[pid[:N]] = deg
    dinv_perm = np.zeros(NPAD, np.float32)
    dinv_perm[pid[:N]] = dinv
    x_perm = np.zeros((NPAD, D), np.float32)
    x_perm[pid[:N]] = x

    # zero rows for ELL padding: any zero-degree node (t stays 0 forever).
    zrow_candidates = np.where(deg_perm == 0)[0]
    assert zrow_candidates.size > 0
    zrow = int(_trow(np.array([zrow_candidates[0]]))[0])

    # ---- ELL structure ----
    s_p = pid[src]
    d_p = pid[dst]
    d_core = d_p // NPC
    d_rem = d_p % NPC
    d_lane = d_rem // TPC
    d_slot = d_rem % TPC

    # degree of node at (c, q, i) = deg_perm[c*NPC + q*TPC + i]
    dg = deg_perm.reshape(C, P, TPC)                          # [c, q, i]
    k_per_slot = dg.max(axis=(0, 1)).astype(np.int64)         # [TPC] max over cores+lanes
    # because ranks are degree-sorted and dealt round-robin, k is tight
    colbase = np.zeros(TPC + 1, np.int64)
    colbase[1:] = np.cumsum(k_per_slot)
    ncols = int(colbase[TPC])

    idx_np = np.full((C, P, ncols), zrow, np.int32)           # pad -> zero row
    # place each edge: order within (node) arbitrary
    eorder = np.lexsort((s_p, d_p))                           # group edges by dst node
    sd = d_p[eorder]
    ss = _trow(s_p[eorder]).astype(np.int32)                  # table rows of sources
    # position within node group
    grp_start = np.searchsorted(sd, d_p[eorder], side="left")
    j_in_node = np.arange(sd.size) - grp_start
    ec = (sd // NPC).astype(np.int64)
    erem = sd % NPC
    eq = erem // TPC
    eslot = erem % TPC
    idx_np[ec, eq, colbase[eslot] + j_in_node] = ss

    dinv_cols = dinv_perm.reshape(C, P, TPC).copy()           # [c, q, i]

    t0_pid = dinv_perm[:, None] * x_perm                      # [NPAD, D] by pid
    t0 = np.empty_like(t0_pid)
    t0[_trow(np.arange(NPAD))] = t0_pid                       # reorder to table rows

    x_need = np.transpose(
        x_perm.reshape(C, P, TPC, D)[:, :, :need_slots, :], (0, 1, 2, 3)
    ).reshape(C, P, need_slots * D).copy()                    # [c, q, i*D]

    # ---- final-stage sample indices into emb_cat ----
    # emb_cat row for node (c, q, i<need_slots) = c*(P*need_slots) + q*need_slots + i
    def emb_row(node_pid):
        c = node_pid // NPC
        rem = node_pid % NPC
        q = rem // TPC
        i = rem % TPC
        assert np.all(i < need_slots), "needed node outside needed slots"
        return c * (P * need_slots) + q * need_slots + i

    u_p = pid[user]
    p_p = pid[pos + NU]
    n_p = pid[neg + NU]
    samp_idx = np.zeros((C, P, 3 * SCOL), np.int32)
    for c in range(C):
        sl = slice(c * BPC, (c + 1) * BPC)
        for blk, arr in enumerate((u_p[sl], p_p[sl], n_p[sl])):
            rows = emb_row(arr)                                # [BPC]
            s = np.arange(BPC)
            samp_idx[c, s % P, blk * SCOL + s // P] = rows

    return dict(
        t0=t0, idx=idx_np, dinv_cols=dinv_cols, dinv2_cols=dinv_cols * dinv_cols,
        x_need=x_need, samp_idx=samp_idx, k_per_slot=k_per_slot, colbase=colbase,
        ncols=ncols, need_slots=need_slots, mid_slots=mid_slots, zrow=zrow,
    )


def _build(pp):
    """Build the Bass program (shared by all 8 cores)."""
    k_per_slot = pp["k_per_slot"]
    colbase = pp["colbase"]
    ncols = pp["ncols"]
    NS = pp["need_slots"]
    MS = pp["mid_slots"]
    ZR = pp["zrow"]
    f32 = mybir.dt.float32
    i32 = mybir.dt.int32

    nc = bass.Bass()
    t0 = nc.dram_tensor("t0", [NPAD, D], f32, kind="ExternalInput")
    idx = nc.dram_tensor("idx", [P, ncols], i32, kind="ExternalInput")
    dinvc = nc.dram_tensor("dinvc", [P, TPC], f32, kind="ExternalInput")
    dinv2c = nc.dram_tensor("dinv2c", [P, TPC], f32, kind="ExternalInput")
    x_need = nc.dram_tensor("x_need", [P, NS * D], f32, kind="ExternalInput")
    samp = nc.dram_tensor("samp", [P, 3 * SCOL], i32, kind="ExternalInput")
    out_ls = nc.dram_tensor("out_ls", [P, SCOL], f32, kind="ExternalOutput")
    out_reg = nc.dram_tensor("out_reg", [P, SCOL], f32, kind="ExternalOutput")

    rg = [list(range(C))]

    with tile.TileContext(nc) as tc:
        with (
            tc.tile_pool(name="const", bufs=1) as cpool,
            tc.tile_pool(name="gath", bufs=14) as gpool,
            tc.tile_pool(name="work", bufs=8) as wpool,
            tc.tile_pool(name="dram", bufs=1, space="DRAM") as dpool,
        ):
            idx_sb = cpool.tile([P, ncols], i32)
            nc.sync.dma_start(out=idx_sb[:], in_=idx[:])
            dinv_sb = cpool.tile([P, TPC], f32)
            nc.sync.dma_start(out=dinv_sb[:], in_=dinvc[:])
            dinv2_sb = cpool.tile([P, TPC], f32)
            nc.sync.dma_start(out=dinv2_sb[:], in_=dinv2c[:])
            acc_sb = cpool.tile([P, NS * D], f32)
            nc.sync.dma_start(out=acc_sb[:], in_=x_need[:])
            tst_sb = cpool.tile([P, TPC * D], f32)
            nc.vector.memset(tst_sb[:], 0.0)
            zpad = cpool.tile([P, D], f32)
            nc.vector.memset(zpad[:], 0.0)
            samp_sb = cpool.tile([P, 3 * SCOL], i32)
            nc.sync.dma_start(out=samp_sb[:], in_=samp[:])

            ag_out_prev = None

            def emit_ag_chunk(layer, ag_out, ci):
                lo, hi = BNDS[ci], BNDS[ci + 1]
                w = hi - lo
                base = C * P * lo
                ag_in = dpool.tile([P * w, D], f32, name=f"agin{ci}_{layer}")
                nc.sync.dma_start(
                    out=ag_in[:].rearrange("(q i) d -> q (i d)", q=P),
                    in_=tst_sb[:, lo * D : hi * D],
                )
                nc.gpsimd.collective_compute(
                    "AllGather", mybir.AluOpType.bypass, replica_groups=rg,
                    ins=[ag_in.opt()],
                    outs=[ag_out[base : base + C * P * w].opt()],
                )

            for layer in range(NL):
                last = layer == NL - 1
                table_ap = t0 if layer == 0 else ag_out_prev
                nslots = NS if last else (TPC if layer == 0 else MS)
                if not last:
                    # plain Internal DRAM (not Shared): chunked collectives
                    # write disjoint slices, which the Shared-DRAM single-writer
                    # rule in the tile scheduler would reject.
                    ag_out = dpool.tile([NPAD, D], f32, name=f"agout{layer}")
                if layer == 1:
                    # layer 2 never gathers rows of slots >= MS (l3src sits
                    # below MS), so layer 1 skips chunk 2's AllGather; only the
                    # ELL pad row (zrow, a slot>=116 node) must read as zero.
                    nc.sync.dma_start(
                        out=ag_out[ZR : ZR + 1], in_=zpad[0:1, :]
                    )
                next_chunk = 0
                # chunk AG emits: early enough that each collective overlaps
                # the remaining slots' compute
                emits = [112, 126] if layer == 0 else [98]
                for i in range(nslots):
                    if (
                        not last
                        and next_chunk < len(emits)
                        and i == emits[next_chunk]
                    ):
                        # chunk AllGather overlapped with remaining compute
                        emit_ag_chunk(layer, ag_out, next_chunk)
                        next_chunk += 1
                    k = int(k_per_slot[i])
                    if k == 0:
                        continue
                    gt = gpool.tile([P, k * D], f32, tag="gt", name=f"g{layer}_{i}")
                    for j in range(k):
                        col = int(colbase[i]) + j
                        nc.gpsimd.indirect_dma_start(
                            out=gt[:, j * D : (j + 1) * D],
                            out_offset=None,
                            in_=table_ap[:],
                            in_offset=bass.IndirectOffsetOnAxis(
                                ap=idx_sb[:, col : col + 1], axis=0
                            ),
                        )
                    # contiguous pairwise-tree reduction over the k slots,
                    # result lands in gt[:, :D]
                    width = k
                    while width > 1:
                        half = width // 2
                        nc.vector.tensor_tensor(
                            out=gt[:, : half * D],
                            in0=gt[:, : half * D],
                            in1=gt[:, half * D : 2 * half * D],
                            op=mybir.AluOpType.add,
                        )
                        if width % 2:
                            nc.vector.tensor_tensor(
                                out=gt[:, :D], in0=gt[:, :D],
                                in1=gt[:, (width - 1) * D : width * D],
                                op=mybir.AluOpType.add,
                            )
                        width = half
                    r_ap = gt[:, :D]
                    if i < NS:
                        h = wpool.tile([P, D], f32, tag="h", name=f"h{layer}_{i}")
                        nc.vector.tensor_scalar(
                            out=h[:], in0=r_ap, scalar1=dinv_sb[:, i : i + 1],
                            scalar2=None, op0=mybir.AluOpType.mult,
                        )
                        nc.vector.tensor_tensor(
                            out=acc_sb[:, i * D : (i + 1) * D],
                            in0=acc_sb[:, i * D : (i + 1) * D],
                            in1=h[:], op=mybir.AluOpType.add,
                        )
                        if not last:
                            nc.vector.tensor_scalar(
                                out=tst_sb[:, i * D : (i + 1) * D],
                                in0=h[:], scalar1=dinv_sb[:, i : i + 1],
                                scalar2=None, op0=mybir.AluOpType.mult,
                            )
                    elif not last:
                        nc.vector.tensor_scalar(
                            out=tst_sb[:, i * D : (i + 1) * D],
                            in0=r_ap, scalar1=dinv2_sb[:, i : i + 1],
                            scalar2=None, op0=mybir.AluOpType.mult,
                        )
                if not last:
                    # remaining chunks (incl. the small final one); layer 1
                    # skips chunk 2 entirely (never read by layer 2)
                    for ci in range(next_chunk, len(BNDS) - 1):
                        if layer == 1 and ci == 2:
                            continue
                        emit_ag_chunk(layer, ag_out, ci)
                    ag_out_prev = ag_out

            # ---- final loss stage ----
            accd = dpool.tile([P * NS, D], f32, name="accd")
            nc.sync.dma_start(
                out=accd[:].rearrange("(q i) d -> q (i d)", q=P), in_=acc_sb[:]
            )
            emb_cat = dpool.tile([C * P * NS, D], f32, addr_space="Shared", name="embcat")
            nc.gpsimd.collective_compute(
                "AllGather", mybir.AluOpType.bypass, replica_groups=rg,
                ins=[accd.opt()], outs=[emb_cat.opt()],
            )
            sg = cpool.tile([P, 3 * SCOL * D], f32)
            for col in range(3 * SCOL):
                nc.gpsimd.indirect_dma_start(
                    out=sg[:, col * D : (col + 1) * D],
                    out_offset=None,
                    in_=emb_cat[:],
                    in_offset=bass.IndirectOffsetOnAxis(
                        ap=samp_sb[:, col : col + 1], axis=0
                    ),
                )
            W = SCOL * D
            u_ap = sg[:, 0:W]
            p_ap = sg[:, W : 2 * W]
            n_ap = sg[:, 2 * W : 3 * W]
            diff = cpool.tile([P, W], f32)
            nc.vector.tensor_tensor(out=diff[:], in0=p_ap, in1=n_ap,
                                    op=mybir.AluOpType.subtract)
            nc.vector.tensor_tensor(out=diff[:], in0=diff[:], in1=u_ap,
                                    op=mybir.AluOpType.mult)
            dots = cpool.tile([P, SCOL], f32)
            nc.vector.reduce_sum(
                out=dots[:], in_=diff[:].rearrange("p (s d) -> p s d", d=D),
                axis=mybir.AxisListType.X,
            )
            ls = cpool.tile([P, SCOL], f32)
            # log_sigmoid(z) = ln(sigmoid(z)), z = dots/16; host negates.
            nc.scalar.activation(
                out=ls[:], in_=dots[:],
                func=mybir.ActivationFunctionType.Sigmoid, scale=1.0 / 16.0,
            )
            nc.scalar.activation(
                out=ls[:], in_=ls[:], func=mybir.ActivationFunctionType.Ln,
            )
            nc.sync.dma_start(out=out_ls[:], in_=ls[:])

            sq = cpool.tile([P, W], f32)
            nc.vector.tensor_tensor(out=sq[:], in0=u_ap, in1=u_ap,
                                    op=mybir.AluOpType.mult)
            tmp = cpool.tile([P, W], f32)
            nc.vector.tensor_tensor(out=tmp[:], in0=p_ap, in1=p_ap,
                                    op=mybir.AluOpType.mult)
            nc.vector.tensor_tensor(out=sq[:], in0=sq[:], in1=tmp[:],
                                    op=mybir.AluOpType.add)
            nc.vector.tensor_tensor(out=tmp[:], in0=n_ap, in1=n_ap,
                                    op=mybir.AluOpType.mult)
            nc.vector.tensor_tensor(out=sq[:], in0=sq[:], in1=tmp[:],
                                    op=mybir.AluOpType.add)
            regs = cpool.tile([P, SCOL], f32)
            nc.vector.reduce_sum(
                out=regs[:], in_=sq[:].rearrange("p (s d) -> p s d", d=D),
                axis=mybir.AxisListType.X,
            )
            nc.sync.dma_start(out=out_reg[:], in_=regs[:])

    _split_multi_waits(nc)
    return nc


def kernel(Gu, Gi, edge_user, edge_item, user, pos, neg, _trace=False):
    pp = _prep(Gu, Gi, edge_user, edge_item, user, pos, neg)
    nc = _build(pp)
    in_maps = [
        {
            "t0": np.ascontiguousarray(pp["t0"]),
            "idx": np.ascontiguousarray(pp["idx"][c]),
            "dinvc": np.ascontiguousarray(pp["dinv_cols"][c]),
            "dinv2c": np.ascontiguousarray(pp["dinv2_cols"][c]),
            "x_need": np.ascontiguousarray(pp["x_need"][c]),
            "samp": np.ascontiguousarray(pp["samp_idx"][c]),
        }
        for c in range(C)
    ]
    res = run_bass_kernel_spmd(nc, in_maps, core_ids=list(range(C)), trace=_trace)
    ls = np.stack([res.results[c]["out_ls"] for c in range(C)])     # [C, P, SCOL]
    rg = np.stack([res.results[c]["out_reg"] for c in range(C)])
    mf = -float(np.mean(ls.astype(np.float64)))
    reg = LW * 0.5 * float(np.sum(rg.astype(np.float64))) / 16.0 / B
    out = np.float32(mf + reg)
    if _trace:
        return out, res
    return out

